# revision 4
# baseline (speedup 1.0000x reference)
"""Multi-head attention (B=2, L=2048, D=1024, H=16) on 8 trn2 NeuronCores.

Sharding: tensor-parallel over heads — 2 heads per core. Each core computes
q/k/v projections for its 2 heads, the attention for those heads, and a
row-parallel partial of the output projection. The host sums the 8 partials
(the "all-reduce") and adds the biases that were folded out of the device
kernel (bv folded through Wo, plus bo).

Device layout notes (everything transposed, feature-major):
  xt   [D, R]      : X.T where X = query.reshape(R, D), R = B*L = 4096
  qt/kt[128, R]    : projected q/k, partitions = 2 heads x 64 head-dims
  va_h [128, R]    : per k-row-tile [128, 128] blocks [v_h | ones] used as
                     PV stationary operand; the ones columns make the PV
                     matmul also produce the softmax denominator.
  logitsT [k, q]   : softmax along the partition axis is avoided entirely —
                     exp() needs no max-subtraction (logits ~ N(0, 0.33^2))
                     and the sum comes out of the PV matmul's ones columns.
"""

import numpy as np
import ml_dtypes

import concourse.bass as bass
import concourse.mybir as mybir
import concourse.tile as tile
from concourse import bacc
from concourse.bass_utils import run_bass_kernel_spmd

B, L, D, H = 2, 2048, 1024, 16
HD = D // H              # 64 head dim
N_CORES = 8
HPC = H // N_CORES       # 2 heads per core
DK = HPC * HD            # 128 local qkv feature dim
R = B * L                # 4096 rows
KC = D // 128            # 8 contraction chunks for the projections
QB = 512                 # q block (psum bank width in fp32)
NQB = L // QB            # 4 q blocks per batch
NKT = L // 128           # 16 k tiles per batch
NRT = R // 128           # 32 row tiles
SCALE = HD ** -0.5

BF16 = mybir.dt.bfloat16
F32 = mybir.dt.float32
Act = mybir.ActivationFunctionType

_BF16_NP = ml_dtypes.bfloat16


def _body(tc, nc, xt_d, wqt_d, wkt_d, wvt_d, bq_d, bk_d, wot_d, out_d):
    with (
        tc.tile_pool(name="consts", bufs=1) as constp,
        tc.tile_pool(name="bigs", bufs=1) as bigs,
        tc.tile_pool(name="work", bufs=1) as work,
        tc.tile_pool(name="outst", bufs=3) as outst,
        tc.tile_pool(name="psum", bufs=1, space="PSUM") as psum,
    ):
        # ---- load weights / biases ----
        wq_sb = constp.tile([128, KC, DK], BF16)
        wk_sb = constp.tile([128, KC, DK], BF16)
        wv_sb = constp.tile([128, KC, DK], BF16)
        wot_sb = constp.tile([DK, D], BF16)
        bq_sb = constp.tile([DK, 1], F32)
        bk_sb = constp.tile([DK, 1], F32)
        nc.sync.dma_start(out=wq_sb, in_=wqt_d[:])
        nc.sync.dma_start(out=wk_sb, in_=wkt_d[:])
        nc.sync.dma_start(out=wv_sb, in_=wvt_d[:])
        nc.sync.dma_start(out=wot_sb, in_=wot_d[:])
        nc.sync.dma_start(out=bq_sb, in_=bq_d[:])
        nc.sync.dma_start(out=bk_sb, in_=bk_d[:])

        # ---- load X.T ----
        xt_sb = []
        for c in range(KC):
            t = bigs.tile([128, R], BF16, name=f"xt{c}")
            nc.sync.dma_start(out=t, in_=xt_d[c * 128 : (c + 1) * 128, :])
            xt_sb.append(t)

        qt = bigs.tile([DK, R], BF16)
        kt = bigs.tile([DK, R], BF16)
        yt = bigs.tile([DK, R], BF16)
        # va[h]: per 128-row k tile, cols [h*64, h*64+64) hold v_h, the other
        # 64 cols stay at the memset value 1.0 (denominator generator).
        va = [bigs.tile([128, R], BF16, name=f"va{h}") for h in range(HPC)]
        for h in range(HPC):
            nc.gpsimd.memset(va[h][:], 1.0)

        # ---- q/k projections (feature-major: psum = W_chunk.T @ XT_chunk) ----
        for nchunk in range(R // QB):
            cols = slice(nchunk * QB, (nchunk + 1) * QB)
            for wsb, bsb, dest in ((wk_sb, bk_sb, kt), (wq_sb, bq_sb, qt)):
                ps = psum.tile([128, QB], F32, tag="io", bufs=3, name="ps_proj")
                for c in range(KC):
                    nc.tensor.matmul(
                        ps,
                        lhsT=wsb[:, c, :],
                        rhs=xt_sb[c][:, cols],
                        start=(c == 0),
                        stop=(c == KC - 1),
                    )
                nc.scalar.activation(
                    out=dest[:, cols], in_=ps, func=Act.Identity, bias=bsb, scale=1.0
                )

        # ---- v projection (row-major: psum = XT_chunk.T @ Wv_chunk) ----
        for t in range(NRT):
            ps = psum.tile([128, QB], F32, tag="io", bufs=3, name="ps_v")
            for c in range(KC):
                nc.tensor.matmul(
                    ps[:, 0:DK],
                    lhsT=xt_sb[c][:, t * 128 : (t + 1) * 128],
                    rhs=wv_sb[:, c, :],
                    start=(c == 0),
                    stop=(c == KC - 1),
                )
            for h in range(HPC):
                nc.scalar.copy(
                    out=va[h][:, t * 128 + h * HD : t * 128 + (h + 1) * HD],
                    in_=ps[:, h * HD : (h + 1) * HD],
                )

        # ---- attention ----
        for b in range(B):
            for qb in range(NQB):
                qcols = slice(b * L + qb * QB, b * L + (qb + 1) * QB)
                pv0 = psum.tile([128, QB], F32, tag="pv", bufs=2, name="pv0")
                pv1 = psum.tile([128, QB], F32, tag="pv", bufs=2, name="pv1")
                for k in range(NKT):
                    tg = b * NKT + k
                    kcols = slice(b * L + k * 128, b * L + (k + 1) * 128)
                    pl0 = psum.tile([128, QB], F32, tag="lg", bufs=3, name="pl0")
                    pl1 = psum.tile([128, QB], F32, tag="lg", bufs=3, name="pl1")
                    # two heads packed into disjoint PE row groups (K=64 each)
                    nc.tensor.matmul(
                        pl0, lhsT=kt[0:HD, kcols], rhs=qt[0:HD, qcols],
                        start=True, stop=True,
                    )
                    nc.tensor.matmul(
                        pl1, lhsT=kt[HD:DK, kcols], rhs=qt[HD:DK, qcols],
                        start=True, stop=True,
                    )
                    e0 = work.tile([128, QB], BF16, tag="exp", bufs=4, name="e0")
                    e1 = work.tile([128, QB], BF16, tag="exp", bufs=4, name="e1")
                    nc.scalar.activation(out=e0, in_=pl0, func=Act.Exp, scale=SCALE)
                    nc.scalar.activation(out=e1, in_=pl1, func=Act.Exp, scale=SCALE)
                    nc.tensor.matmul(
                        pv0, lhsT=va[0][:, tg * 128 : (tg + 1) * 128], rhs=e0,
                        start=(k == 0), stop=(k == NKT - 1),
                    )
                    nc.tensor.matmul(
                        pv1, lhsT=va[1][:, tg * 128 : (tg + 1) * 128], rhs=e1,
                        start=(k == 0), stop=(k == NKT - 1),
                    )
                # pv0 = [Yun_h0 (p 0:64); denom_h0 (p 64:128)]
                # pv1 = [denom_h1 (p 0:64); Yun_h1 (p 64:128)]
                rsw = work.tile([128, QB], F32, tag="rsw", bufs=2, name="rsw")
                nc.vector.reciprocal(out=rsw[HD:128, :], in_=pv0[HD:128, :])
                nc.vector.reciprocal(out=rsw[0:HD, :], in_=pv1[0:HD, :])
                # swap halves across partitions (DMA is the cross-lane engine)
                rr = work.tile([128, QB], F32, tag="rr", bufs=2, name="rr")
                nc.sync.dma_start(out=rr[0:HD, :], in_=rsw[HD:128, :])
                nc.sync.dma_start(out=rr[HD:128, :], in_=rsw[0:HD, :])
                nc.vector.tensor_mul(
                    out=yt[0:HD, qcols], in0=pv0[0:HD, :], in1=rr[0:HD, :]
                )
                nc.vector.tensor_mul(
                    out=yt[HD:DK, qcols], in0=pv1[HD:DK, :], in1=rr[HD:DK, :]
                )

        # ---- output projection partial: out[t*128:, :] = YT_chunk.T @ WoT ----
        for t in range(NRT):
            ycols = slice(t * 128, (t + 1) * 128)
            po0 = psum.tile([128, QB], F32, tag="io", bufs=3, name="po0")
            po1 = psum.tile([128, QB], F32, tag="io", bufs=3, name="po1")
            nc.tensor.matmul(
                po0, lhsT=yt[:, ycols], rhs=wot_sb[:, 0:QB], start=True, stop=True
            )
            nc.tensor.matmul(
                po1, lhsT=yt[:, ycols], rhs=wot_sb[:, QB:D], start=True, stop=True
            )
            ost = outst.tile([128, D], F32, name="ost")
            nc.vector.tensor_copy(out=ost[:, 0:QB], in_=po0)
            nc.vector.tensor_copy(out=ost[:, QB:D], in_=po1)
            nc.sync.dma_start(out=out_d[t * 128 : (t + 1) * 128, :], in_=ost)


def build_bass():
    nc = bacc.Bacc("TRN2", target_bir_lowering=False, debug=False)
    xt_d = nc.dram_tensor("xt", [D, R], BF16, kind="ExternalInput")
    wqt_d = nc.dram_tensor("wqt", [128, KC, DK], BF16, kind="ExternalInput")
    wkt_d = nc.dram_tensor("wkt", [128, KC, DK], BF16, kind="ExternalInput")
    wvt_d = nc.dram_tensor("wvt", [128, KC, DK], BF16, kind="ExternalInput")
    bq_d = nc.dram_tensor("bq", [DK, 1], F32, kind="ExternalInput")
    bk_d = nc.dram_tensor("bk", [DK, 1], F32, kind="ExternalInput")
    wot_d = nc.dram_tensor("wot", [DK, D], BF16, kind="ExternalInput")
    out_d = nc.dram_tensor("out", [R, D], F32, kind="ExternalOutput")
    with tile.TileContext(nc) as tc:
        _body(tc, nc, xt_d, wqt_d, wkt_d, wvt_d, bq_d, bk_d, wot_d, out_d)
    nc.compile()
    return nc


_NC = None


def _get_nc():
    global _NC
    if _NC is None:
        _NC = build_bass()
    return _NC


def prepare(inputs):
    """Full inputs -> (per-core in_maps, host-side bias constant)."""
    q = np.asarray(inputs["query"], np.float32)
    Wq = np.asarray(inputs["Wq"], np.float32)
    Wk = np.asarray(inputs["Wk"], np.float32)
    Wv = np.asarray(inputs["Wv"], np.float32)
    Wo = np.asarray(inputs["Wo"], np.float32)
    bq = np.asarray(inputs["bq"], np.float32)
    bk = np.asarray(inputs["bk"], np.float32)
    bv = np.asarray(inputs["bv"], np.float32)
    bo = np.asarray(inputs["bo"], np.float32)

    X = q.reshape(R, D)
    xt = np.ascontiguousarray(X.T).astype(_BF16_NP)

    def wslice(W, hs):
        # W[hs].T laid out [p, chunk, m]: in-feat within chunk, chunk, out-feat
        return np.ascontiguousarray(
            W[hs, :].T.reshape(KC, 128, DK).transpose(1, 0, 2)
        ).astype(_BF16_NP)

    in_maps = []
    const = bo.astype(np.float64).copy()
    for c in range(N_CORES):
        hs = slice(c * DK, (c + 1) * DK)
        const += Wo[:, hs].astype(np.float64) @ bv[hs].astype(np.float64)
        in_maps.append(
            {
                "xt": xt,
                "wqt": wslice(Wq, hs),
                "wkt": wslice(Wk, hs),
                "wvt": wslice(Wv, hs),
                "bq": np.ascontiguousarray(bq[hs].reshape(DK, 1)),
                "bk": np.ascontiguousarray(bk[hs].reshape(DK, 1)),
                "wot": np.ascontiguousarray(Wo[:, hs].T).astype(_BF16_NP),
            }
        )
    return in_maps, const


def finish(results, const):
    acc = np.zeros((R, D), np.float64)
    for r in results:
        acc += np.asarray(r["out"], np.float64)
    acc += const[None, :]
    return acc.astype(np.float32).reshape(B, L, D)


def run(in_maps, trace=False, **kwargs):
    nc = _get_nc()
    return run_bass_kernel_spmd(nc, in_maps, list(range(N_CORES)), trace=trace, **kwargs)


def kernel(**inputs):
    in_maps, const = prepare(inputs)
    res = run(in_maps)
    return finish(res.results, const)
